# revision 1
# baseline (speedup 1.0000x reference)
"""CRF loss (forward-algorithm partition + gold-path score) on 8 trn2 NeuronCores.

Strategy
--------
Denominator (log-partition, ~99.6% of reference FLOPs): the logsumexp scan is a
matmul in exp space:  alpha_t = log( exp(trans).T @ exp(alpha_{t-1}) ) + e_t.
Keeping the state in exp space, each step is one PE matmul with constant
weights W = exp(trans - C) plus one DVE elementwise multiply by exp(e_t)
(computed on ACT off the critical path). The constant per-step decay e^-C
keeps the f32/bf16 state centered (measured log-range ~[-10, +5] for this
data) with zero per-step rescaling cost; the exact correction +2*255*C is
applied in log space at the end.

Sharding: batch 1024 -> 4 shards x 256; time 512 -> forward half (t=0..255)
and backward half (t=511..256, reversed) = 8 cores, meeting in the middle:
  log Z_b = log( F[:,b].T @ exp(trans) @ R[:,b] ) + 510*C
where F = fwd exp-state after t=255, R = bwd exp-state after t=256. The tiny
[64x64x256] bridge per shard is done on host in f64 (stability), along with
the O(B) final add/sum — everything O(L*B*T) runs on device.

Numerator: gold-path gathers (pure indexing) are marshaled on host
(np.take_along_axis / fancy indexing); their O(L*B) reduction runs on device.

Host-side work is indexing/layout/dtype marshaling only, plus the O(B)
finalize.
"""

import os

import ml_dtypes
import numpy as np

import concourse.bass as bass
import concourse.bacc as bacc
import concourse.mybir as mybir
from concourse.bass_utils import run_bass_kernel_spmd
from concourse.tile import TileContext

BF16 = ml_dtypes.bfloat16

L, B, T = 512, 1024, 64
NCORES = 8
NSHARDS = 4                  # batch shards; cores 0-3 fwd, 4-7 bwd
BL = B // NSHARDS            # 256 batch columns per core
S = int(os.environ.get("CRF_STEPS", str(L // 2)))   # tiles per core (256)
NCH = int(os.environ.get("CRF_NCHAINS", "2"))       # independent chains per core
G = 2                        # tag-groups stacked on partitions (blockdiag weights)
P = G * T                    # 128 partitions
CW = BL // (G * NCH)         # free columns per chain tile
SB = int(os.environ.get("CRF_SB", "8"))             # emission steps per DMA chunk
DECAY = 4.66                 # per-matmul-step exp-space decay (keeps state centered)

_COMPILED = {}
LAST_RUN = {}


def _build_nc():
    nc = bacc.Bacc("TRN2", target_bir_lowering=False, debug=False)
    f32 = mybir.dt.float32
    bf16 = mybir.dt.bfloat16

    assert S % SB == 0 or S < SB
    nch_chunks = max(1, S // SB)
    emi = nc.dram_tensor(
        "emi", [nch_chunks, P, min(SB, S) * (BL // G)], bf16, kind="ExternalInput"
    )
    wmat = nc.dram_tensor("wmat", [P, P], bf16, kind="ExternalInput")
    biasv = nc.dram_tensor("biasv", [P, 1], f32, kind="ExternalInput")
    nums = nc.dram_tensor("nums", [2, 128, 2 * S], f32, kind="ExternalInput")

    fstate = nc.dram_tensor("fstate", [P, BL // G], bf16, kind="ExternalOutput")
    numpart = nc.dram_tensor("numpart", [2, 128, 1], f32, kind="ExternalOutput")

    with TileContext(nc) as tc:
        with (
            tc.tile_pool(name="consts", bufs=1) as consts,
            tc.tile_pool(name="emi", bufs=int(os.environ.get("CRF_EMI_BUFS", "4"))) as emi_pool,
            tc.tile_pool(name="ep", bufs=int(os.environ.get("CRF_EMI_BUFS", "4"))) as ep_pool,
            tc.tile_pool(name="state", bufs=int(os.environ.get("CRF_STATE_BUFS", "3"))) as p_pool,
            tc.tile_pool(name="psum", bufs=int(os.environ.get("CRF_PSUM_BUFS", "2")), space="PSUM") as psum_pool,
            tc.tile_pool(name="numr", bufs=2) as num_pool,
        ):
            w_tile = consts.tile([P, P], bf16)
            nc.sync.dma_start(out=w_tile[:], in_=wmat[:, :])
            bias_tile = consts.tile([P, 1], f32)
            nc.sync.dma_start(out=bias_tile[:], in_=biasv[:, :])

            # numerator reduction: two [128, 2S] slabs -> row sums
            for h in range(2):
                ntile = num_pool.tile([128, 2 * S], f32, tag="ntile")
                nc.sync.dma_start(out=ntile[:], in_=nums[h])
                nred = num_pool.tile([128, 1], f32, tag="nred")
                nc.vector.reduce_sum(
                    out=nred[:], in_=ntile[:], axis=mybir.AxisListType.X
                )
                nc.gpsimd.dma_start(out=numpart[h], in_=nred[:])

            # main exp-space scan
            p_prev = [None] * NCH
            echunk, epchunk = None, None
            W = BL // G
            ecw = min(SB, S) * W
            for s in range(S):
                if s % SB == 0:
                    echunk = emi_pool.tile([P, ecw], bf16, tag="et")
                    nc.sync.dma_start(out=echunk[:], in_=emi[s // SB])
                    epchunk = ep_pool.tile([P, ecw], bf16, tag="ep")
                    nc.scalar.activation(
                        epchunk[:], echunk[:], mybir.ActivationFunctionType.Exp
                    )
                et = echunk[:, (s % SB) * W : (s % SB + 1) * W]
                ep = epchunk[:, (s % SB) * W : (s % SB + 1) * W]
                if s == 0:
                    for cn in range(NCH):
                        p0 = p_pool.tile([P, CW], bf16, tag=f"p{cn}")
                        nc.scalar.activation(
                            p0[:],
                            et[:, cn * CW : (cn + 1) * CW],
                            mybir.ActivationFunctionType.Exp,
                            bias=bias_tile[:],
                        )
                        p_prev[cn] = p0
                    continue
                for cn in range(NCH):
                    m = psum_pool.tile([P, CW], f32, tag=f"m{cn}")
                    nc.tensor.matmul(
                        m[:], w_tile[:], p_prev[cn][:], start=True, stop=True
                    )
                    pn = p_pool.tile([P, CW], bf16, tag=f"p{cn}")
                    nc.vector.tensor_tensor(
                        out=pn[:],
                        in0=m[:],
                        in1=ep[:, cn * CW : (cn + 1) * CW],
                        op=mybir.AluOpType.mult,
                    )
                    p_prev[cn] = pn

            for cn in range(NCH):
                nc.sync.dma_start(
                    out=fstate[:, cn * CW : (cn + 1) * CW], in_=p_prev[cn][:]
                )
    nc.compile()
    return nc


def kernel(emissions, tags, mask, start_transitions, end_transitions, transitions):
    emissions = np.asarray(emissions, dtype=np.float32)          # (L, B, T)
    tags = np.asarray(tags).astype(np.int64)                     # (L, B)
    mask = np.asarray(mask)
    start_transitions = np.asarray(start_transitions, dtype=np.float32)
    end_transitions = np.asarray(end_transitions, dtype=np.float32)
    transitions = np.asarray(transitions, dtype=np.float32)
    assert bool(mask.all()), "kernel specialized for all-ones mask"

    half = L // 2

    # ---- host marshaling: layout + dtype only ----
    # gold-path gathers (indexing only; reductions happen on device)
    EG = np.take_along_axis(emissions, tags[:, :, None], axis=2)[:, :, 0]  # (L,B)
    TRS = np.zeros((L, B), np.float32)
    TRS[1:] = transitions[tags[:-1], tags[1:]]
    SG = start_transitions[tags[0]]
    ENG = end_transitions[tags[-1]]

    def blockdiag(w):
        wb = np.zeros((P, P), np.float32)
        wb[:T, :T] = w
        wb[T:, T:] = w
        return wb.astype(BF16)

    Wf = blockdiag(np.exp(transitions - DECAY))       # fwd lhsT [cur, next] x2
    Wb = blockdiag(np.exp(transitions.T - DECAY))     # bwd lhsT [next, cur] x2
    bias_f = np.concatenate([start_transitions, start_transitions]).reshape(P, 1)
    bias_b = np.concatenate([end_transitions, end_transitions]).reshape(P, 1)

    def stack_emi(slab):
        # slab (S, 256, 64) f32, b_local = 128c + 64g + j -> [chunk, 64g+k, (s%SB, 64c+j)]
        r = slab.reshape(S, 2, G, T, T)               # (S, c, g, j, k)
        r = r.transpose(0, 2, 4, 1, 3)                # (S, g, k, c, j)
        r = r.reshape(S, P, BL // G)
        sb = min(SB, S)
        r = r.reshape(S // sb, sb, P, BL // G).transpose(0, 2, 1, 3)
        return np.ascontiguousarray(
            r.reshape(S // sb, P, sb * (BL // G))
        ).astype(BF16)

    in_maps = []
    for core in range(NCORES):
        sh = core % NSHARDS
        is_bwd = core >= NSHARDS
        bsl = slice(sh * BL, (sh + 1) * BL)
        if not is_bwd:
            emi_c = stack_emi(emissions[:half, bsl][:S])
            numc = (EG[:half, bsl], TRS[:half, bsl])
        else:
            emi_c = stack_emi(emissions[half:, bsl][::-1][:S])
            numc = (EG[half:, bsl], TRS[half:, bsl])
        # nums layout: [half-of-shard h, 128 rows, EG(S) || TRS(S)]
        nums_c = np.empty((2, 128, 2 * S), np.float32)
        for h in range(2):
            rows = slice(h * 128, (h + 1) * 128)
            nums_c[h, :, :S] = numc[0][:S, rows].T
            nums_c[h, :, S:] = numc[1][:S, rows].T
        in_maps.append(
            {
                "emi": emi_c,
                "wmat": Wb if is_bwd else Wf,
                "biasv": bias_b if is_bwd else bias_f,
                "nums": nums_c,
            }
        )

    if "nc" not in _COMPILED:
        _COMPILED["nc"] = _build_nc()
    res = run_bass_kernel_spmd(
        _COMPILED["nc"],
        in_maps,
        list(range(NCORES)),
        trace=bool(int(os.environ.get("CRF_TRACE", "0"))),
    )
    LAST_RUN["exec_time_ns"] = res.exec_time_ns
    LAST_RUN["profile_json"] = res.profile_json
    outs = res.results

    # ---- host finalize: tiny f64 bridge + O(B) sums ----
    def unstack(fs):
        # [64g+k, 64c+j] -> [k, 128c+64g+j]
        r = fs.reshape(G, T, 2, T).transpose(1, 2, 0, 3)
        return np.ascontiguousarray(r.reshape(T, BL))

    Texp = np.exp(transitions.astype(np.float64))
    total = 0.0
    for sh in range(NSHARDS):
        F = unstack(outs[sh]["fstate"]).astype(np.float64)            # (T, BL)
        R = unstack(outs[NSHARDS + sh]["fstate"]).astype(np.float64)  # (T, BL)
        z = np.einsum("ib,ij,jb->b", F, Texp, R)
        log_z = np.log(z) + 2 * (S - 1) * DECAY
        bsl = slice(sh * BL, (sh + 1) * BL)
        num = (
            outs[sh]["numpart"].reshape(BL)
            + outs[NSHARDS + sh]["numpart"].reshape(BL)
            + SG[bsl]
            + ENG[bsl]
        )
        total += float((num.astype(np.float64) - log_z).sum())
    return np.float32(total)

